# revision 33
# baseline (speedup 1.0000x reference)
"""BayesLinear (reparameterized Bayesian linear layer) Trainium2 kernel.

Computes  y = x @ (mu + softplus(rho) * eps_w)^T + (b_mu + softplus(b_rho) * b_eps)
for x [8192, 4096], weights [4096, 4096], on 8 NeuronCores.

Sharding: the contraction dim D_IN is split 2-way and out_features 4-way
(2x4 grid over 8 cores). Each core computes a partial product
y_part [8192, 1024] = x[:, d_shard] @ W[o_shard, d_shard]^T (+ bias on
d-group 0 only; d-group 1 cores receive zeroed bias inputs so their bias
contribution is exactly 0). The host sums the two d-group partials and
concatenates the four o-shards.

Layout trick: the host uploads the x shard TRANSPOSED (x^T [d, t]) and the
weight shards transposed (W^T [d, o]) so the contraction dim lands on SBUF
partitions directly from DMA — no on-device transposes at all (host-side
np transposes are layout-only sharding work and are not on the device
critical path).

On-device per core:
  - W^T = mu^T + softplus(rho^T)*eps^T computed elementwise per 128-row
    k-tile; softplus(x) = Ln(1*e^x + 1) uses two ACT instructions (Exp and
    Ln live in the same HW activation table set) + 2 DVE tensor-tensor
    ops, output cast to bf16 [128, O] and kept resident (16 tiles).
  - x^T streams in per (k, token-window) as SWDGE cast-DMAs (f32->bf16
    during the transfer) into resident [128, TW] tiles, double-buffered
    by window.
  - TensorE runs 16-deep PSUM accumulation groups (K=16 k-tiles) of bf16
    matmuls, stationary = x^T slab column block (shared by the J=2
    output-chunk matmuls, so LDWEIGHTS is amortized/hidden), moving =
    W^T k-tile [128, 512].
  - Bias is added during PSUM eviction on DVE with bf16 output; y is
    stored bf16 (host upcasts and sums the two d-group partials in f32).
"""

import os
import sys

import numpy as np

for _p in ("/opt/trn_rl_repo", "/root/.axon_site/_ro/trn_rl_repo"):
    if os.path.isdir(_p) and _p not in sys.path:
        sys.path.append(_p)

import concourse.bass as bass  # noqa: E402
import concourse.mybir as mybir  # noqa: E402
import concourse.tile as tile  # noqa: E402
from concourse import bacc, bass_utils  # noqa: E402

P = 128
TOKENS, D_IN, D_OUT = 8192, 4096, 4096
N_CORES = 8
D_SHARDS = 2  # contraction-dim shards
O_SHARDS = 4  # out-features shards
D_LOC = D_IN // D_SHARDS  # 2048
O_LOC = D_OUT // O_SHARDS  # 1024


def build_nc(T=TOKENS, D=D_LOC, O=O_LOC, nf=512, reps=1, variant=("jsplit",), tw=1024, psb=8, xwb=2, ypb=2, wkb=2):
    """Build + compile the per-core SPMD Bass program.

    reps>1 wraps the whole body in an on-device For_i loop (for slope-based
    timing). `variant` holds debug switches for timing experiments:
    "f32y" (f32 output), "no_sp" (skip softplus chain), "no_wphase".
    """
    f32 = mybir.dt.float32
    bf16 = mybir.dt.bfloat16
    alu = mybir.AluOpType
    Exp = mybir.ActivationFunctionType.Exp
    Ln = mybir.ActivationFunctionType.Ln
    K = D // P  # contraction tiles (16)
    NI = T // P  # token slabs (64)
    nf = min(nf, O)
    J = O // nf  # matmul free-dim chunks
    NW = T // tw  # token windows
    SW = tw // P  # slabs per window
    ydt = f32 if "f32y" in variant else bf16

    indt = f32 if "f32in" in variant else bf16
    WC_K = 2  # k-tiles per W chunk (must match _shard_inputs layout)
    NC = K // WC_K

    nc = bacc.Bacc("TRN2", target_bir_lowering=False, debug=False)
    # All device tensors are host-pre-tiled so that every DMA reads/writes
    # per-partition-contiguous blocks (minimal descriptor counts):
    #   x   [P, NW*K*tw]    = x^T tiled as [p][w][k][t_in_w]
    #   w*  [P, NC*WC_K*O]  = W^T tiled as [p][c][kk][o]
    #   y   [P, NW*SW*O]    = y   tiled as [p][w][s][o]  (host re-gathers)
    # x / mu / eps are uploaded bf16 (the kernel rounds them to bf16 anyway);
    # rho stays f32 so softplus sees full input precision.
    # wpk packs (rho fp16 | mu bf16 | eps bf16) per chunk so each W chunk is
    # ONE contiguous DMA: per partition per chunk 3*O f32 words.
    f16 = mybir.dt.float16
    x = nc.dram_tensor("x", [P, T * (D // P)], indt, kind="ExternalInput")
    wpk = nc.dram_tensor("wpk", [P, NC * 3 * O], f32, kind="ExternalInput")
    bmu = nc.dram_tensor("bmu", [O], f32, kind="ExternalInput")
    brho = nc.dram_tensor("brho", [O], f32, kind="ExternalInput")
    beps = nc.dram_tensor("beps", [O], f32, kind="ExternalInput")
    y = nc.dram_tensor("y", [P, (T // P) * O], ydt, kind="ExternalOutput")

    with tile.TileContext(nc) as tc:
        with (
            tc.tile_pool(name="wt", bufs=1) as wtp,
            tc.tile_pool(name="wk", bufs=wkb) as wkp,
            tc.tile_pool(name="bias", bufs=1) as bp,
            tc.tile_pool(name="xs", bufs=xwb) as xsp,
            tc.tile_pool(name="yp", bufs=ypb) as yp,
            tc.tile_pool(name="ps", bufs=psb, space="PSUM") as psp,
            tc.tile_pool(name="dram", bufs=1, space="DRAM") as dramp,
        ):
            def emit_body():
                # ---- W phase: one resident W^T tile [P, K, O] bf16, filled by
                # chunked big DMAs + chunked softplus. With "jsplit" the
                # chunks stream output-half-major so j=0 matmuls can start
                # while the j=1 half of W is still in flight.
                jsplit = "jsplit" in variant
                WT = wtp.tile([P, K, O], bf16, tag="WT")
                # hoist the first x windows ahead of the W phase so they are
                # not queued behind W chunks on the HWDGE rings
                xw_pre = {}
                if "no_x" not in variant:
                    XW0 = K * tw
                    for w in range(min(1, NW)):
                        xw = xsp.tile([P, K, tw], indt, tag="xw", name=f"xw{w}")
                        # window 0 rides SWDGE (otherwise idle) so the ACT
                        # ring only carries W chunks during the startup phase
                        nc.gpsimd.dma_start(
                            xw[:],
                            x[:, w * XW0 : (w + 1) * XW0].rearrange(
                                "p (k t) -> p k t", k=K
                            ),
                        )
                        xw_pre[w] = xw

                if "no_wphase" in variant:
                    nc.gpsimd.memset(WT[:], 0.0)
                else:
                    CH = 3 * O  # packed f32 words per chunk per partition
                    for c in range(NC):
                        cslc = slice(c * WC_K, (c + 1) * WC_K)
                        wst = wkp.tile([P, CH], f32, tag="wst")
                        # alternate HWDGE rings (sync=SP, scalar=ACT)
                        eng = nc.sync if c % 2 == 0 else nc.scalar
                        eng.dma_start(wst[:], wpk[:, c * CH : (c + 1) * CH])
                        rho_c = (
                            wst[:, 0:O]
                            .bitcast(f16)
                            .rearrange("p (k o) -> p k o", k=WC_K)
                        )
                        mu_c = (
                            wst[:, O : 2 * O]
                            .bitcast(bf16)
                            .rearrange("p (k o) -> p k o", k=WC_K)
                        )
                        eps_c = (
                            wst[:, 2 * O : 3 * O]
                            .bitcast(bf16)
                            .rearrange("p (k o) -> p k o", k=WC_K)
                        )
                        if "no_sp" in variant:
                            nc.vector.tensor_add(WT[:, cslc, :], eps_c, mu_c)
                        else:
                            sp = wkp.tile([P, WC_K, O], f32, tag="sp")
                            nc.scalar.activation(sp[:], rho_c, Exp)
                            nc.scalar.activation(sp[:], sp[:], Ln, bias=1.0)
                            nc.vector.tensor_mul(sp[:], sp[:], eps_c)
                            nc.vector.tensor_add(WT[:, cslc, :], sp[:], mu_c)

                # ---- bias on one partition, then broadcast to [P, O].
                # Emitted AFTER the W phase (and on SWDGE for the small rows)
                # so it never delays W chunks on the HWDGE rings; it only has
                # to beat the first eviction, which is ~W-phase-end anyway.
                brow_mu = bp.tile([1, O], f32, tag="bmu")
                nc.gpsimd.dma_start(brow_mu[:], bmu[:][None, :])
                brow_rho = bp.tile([1, O], f32, tag="brho")
                nc.gpsimd.dma_start(brow_rho[:], brho[:][None, :])
                brow_eps = bp.tile([1, O], f32, tag="beps")
                nc.gpsimd.dma_start(brow_eps[:], beps[:][None, :])
                bsp = bp.tile([1, O], f32, tag="bsp")
                nc.scalar.activation(bsp[:], brow_rho[:], Exp)
                nc.scalar.activation(bsp[:], bsp[:], Ln, bias=1.0)
                brow = bp.tile([1, O], f32, tag="brow")
                nc.vector.tensor_mul(brow[:], bsp[:], brow_eps[:])
                nc.vector.tensor_add(brow[:], brow[:], brow_mu[:])
                brow_d = dramp.tile([1, O], f32, tag="browd")
                nc.gpsimd.dma_start(brow_d[:], brow[:])
                bfull = bp.tile([P, O], f32, tag="bfull")
                nc.gpsimd.dma_start(bfull[:], brow_d[:].to_broadcast([P, O]))

                # ---- main loop: windows of tw tokens, slabs of 128
                XW = K * tw  # flat elements per x window per partition
                xw_static = None
                if "no_x" in variant:
                    xw_static = xsp.tile([P, K, tw], bf16, tag="xw")
                    nc.gpsimd.memset(xw_static[:], 0.0)
                for w in range(NW):
                    if xw_static is not None:
                        xw = xw_static
                    elif w in xw_pre:
                        xw = xw_pre[w]
                    else:
                        xw = xsp.tile([P, K, tw], indt, tag="xw", name=f"xw{w}")
                        # HWDGE on the ACT ring so x loads don't queue behind
                        # the W loads / y stores on the SP ring
                        nc.scalar.dma_start(
                            xw[:],
                            x[:, w * XW : (w + 1) * XW].rearrange(
                                "p (k t) -> p k t", k=K
                            ),
                        )
                    ysw = None
                    if "no_y" not in variant:
                        ysw = yp.tile([P, SW, O], ydt, tag="ys", name=f"ys{w}")
                    for s in range(SW):
                        i = w * SW + s
                        csl = slice(s * P, (s + 1) * P)
                        if jsplit:
                            for j in range(J):
                                jsl = slice(j * nf, (j + 1) * nf)
                                ps = psp.tile([P, nf], f32, tag="ps", name=f"ps{i}_{j}")
                                for k in range(K):
                                    nc.tensor.matmul(
                                        ps[:],
                                        xw[:, k, csl],
                                        WT[:, k, jsl],
                                        start=(k == 0),
                                        stop=(k == K - 1),
                                    )
                                if ysw is not None:
                                    nc.vector.tensor_tensor(
                                        ysw[:, s, jsl], ps[:], bfull[:, jsl], alu.add
                                    )
                        else:
                            pss = [
                                psp.tile([P, nf], f32, tag="ps", name=f"ps{i}_{j}")
                                for j in range(J)
                            ]
                            for k in range(K):
                                for j in range(J):
                                    # consecutive matmuls share the stationary
                                    # operand xw[:, k, csl] -> LDWEIGHTS amortized
                                    nc.tensor.matmul(
                                        pss[j][:],
                                        xw[:, k, csl],
                                        WT[:, k, j * nf : (j + 1) * nf],
                                        start=(k == 0),
                                        stop=(k == K - 1),
                                    )
                            if ysw is not None:
                                for j in range(J):
                                    jsl = slice(j * nf, (j + 1) * nf)
                                    nc.vector.tensor_tensor(
                                        ysw[:, s, jsl], pss[j][:], bfull[:, jsl], alu.add
                                    )
                    if ysw is not None:
                        YW = SW * O
                        nc.sync.dma_start(
                            y[:, w * YW : (w + 1) * YW].rearrange(
                                "p (s o) -> p s o", s=SW
                            ),
                            ysw[:],
                        )

            if reps == 1:
                emit_body()
            else:
                with tc.For_i(0, reps, 1):
                    emit_body()

    nc.compile()
    return nc


_NC_CACHE = {}


def _get_nc():
    key = (TOKENS, D_LOC, O_LOC)
    if key not in _NC_CACHE:
        _NC_CACHE[key] = build_nc()
    return _NC_CACHE[key]


TW = 1024  # token-window size (must match build_nc tw default)
SW = TW // P  # slabs per window
NW = TOKENS // TW
WC_K = 2  # k-tiles per W chunk (must match build_nc)
K_LOC = D_LOC // P  # 16
NC_W = K_LOC // WC_K  # 8


def _tile_x(xT_shard, bf16):
    """x^T [D_loc, T] -> [P, NW*K*tw] with per-partition-contiguous windows."""
    a = xT_shard.reshape(K_LOC, P, NW, TW)
    return np.ascontiguousarray(a.transpose(1, 2, 0, 3).astype(bf16)).reshape(
        P, NW * K_LOC * TW
    )


def _pack_w(muT, rhoT, epsT, bf16):
    """W^T shards [D_loc, O_loc] -> one packed [P, NC*3*O] f32 array with
    per-chunk layout (rho fp16 | mu bf16 | eps bf16), per-partition contiguous."""

    def tile4(a):
        return a.reshape(NC_W, WC_K, P, O_LOC).transpose(2, 0, 1, 3)

    ch = WC_K * O_LOC  # 2048 elements (= ch//2 f32 words when 16-bit)
    hw = ch // 2
    pack = np.empty((P, NC_W, 3 * hw), np.float32)
    rho_t = np.ascontiguousarray(tile4(rhoT).astype(np.float16).reshape(P, NC_W, ch))
    mu_t = np.ascontiguousarray(tile4(muT).astype(bf16).reshape(P, NC_W, ch))
    eps_t = np.ascontiguousarray(tile4(epsT).astype(bf16).reshape(P, NC_W, ch))
    pack[:, :, :hw] = rho_t.view(np.float32)
    pack[:, :, hw : 2 * hw] = mu_t.view(np.float32)
    pack[:, :, 2 * hw :] = eps_t.view(np.float32)
    return np.ascontiguousarray(pack.reshape(P, NC_W * 3 * hw))


def _shard_inputs(x, weight_mu, weight_rho, eps_weight, bias_mu, bias_rho, eps_bias):
    """Per-core input maps; x and W shards are uploaded transposed + tiled so
    every device DMA is per-partition contiguous, and x / mu / eps are
    pre-rounded to bf16 (the on-device matmul is bf16)."""
    import ml_dtypes

    bf16 = ml_dtypes.bfloat16
    in_maps = []
    zeros_b = np.zeros(O_LOC, dtype=np.float32)
    xT = {}  # d-group -> tiled x shard (shared across the 4 o-shards)
    for g in range(D_SHARDS):
        dsl = slice(g * D_LOC, (g + 1) * D_LOC)
        xT[g] = _tile_x(x[:, dsl].T, bf16)
    for c in range(N_CORES):
        g, oj = divmod(c, O_SHARDS)
        dsl = slice(g * D_LOC, (g + 1) * D_LOC)
        osl = slice(oj * O_LOC, (oj + 1) * O_LOC)
        im = {
            "x": xT[g],
            "wpk": _pack_w(
                weight_mu[osl, dsl].T,
                weight_rho[osl, dsl].T,
                eps_weight[osl, dsl].T,
                bf16,
            ),
        }
        if g == 0:
            im["bmu"] = np.ascontiguousarray(bias_mu[osl])
            im["brho"] = np.ascontiguousarray(bias_rho[osl])
            im["beps"] = np.ascontiguousarray(eps_bias[osl])
        else:
            im["bmu"] = zeros_b
            im["brho"] = zeros_b
            im["beps"] = zeros_b
        in_maps.append(im)
    return in_maps


def run_sharded(inputs, trace=False, trace_cores=None, tmpdir=None):
    """Run the SPMD kernel on 8 cores; returns (y_full, BassKernelResults)."""
    nc = _get_nc()
    in_maps = _shard_inputs(
        np.asarray(inputs["x"], dtype=np.float32),
        np.asarray(inputs["weight_mu"], dtype=np.float32),
        np.asarray(inputs["weight_rho"], dtype=np.float32),
        np.asarray(inputs["eps_weight"], dtype=np.float32),
        np.asarray(inputs["bias_mu"], dtype=np.float32),
        np.asarray(inputs["bias_rho"], dtype=np.float32),
        np.asarray(inputs["eps_bias"], dtype=np.float32),
    )
    res = bass_utils.run_bass_kernel_spmd(
        nc,
        in_maps,
        core_ids=list(range(N_CORES)),
        trace=trace,
        trace_cores=trace_cores,
        tmpdir=tmpdir,
    )
    def _untile_y(arr):
        # [P, NW*SW*O] -> [T, O]
        a = arr.reshape(P, NW, SW, O_LOC).transpose(1, 2, 0, 3)
        return a.reshape(TOKENS, O_LOC)

    yf = np.empty((TOKENS, D_OUT), dtype=np.float32)
    for oj in range(O_SHARDS):
        osl = slice(oj * O_LOC, (oj + 1) * O_LOC)
        acc = _untile_y(res.results[oj]["y"].astype(np.float32))
        for g in range(1, D_SHARDS):
            acc = acc + _untile_y(
                res.results[g * O_SHARDS + oj]["y"].astype(np.float32)
            )
        yf[:, osl] = acc
    return yf, res


def kernel(**inputs) -> np.ndarray:
    y, _ = run_sharded(inputs, trace=False)
    return y
